# revision 1
# baseline (speedup 1.0000x reference)
"""EdgeOnlyConv GNN message-passing kernel for Trainium2 (8 NeuronCores).

out[e] = concat(x[src[e]], x[dest[e]], edge_attr[e]) @ W.T + b

Strategy (edge-parallel across 8 cores, x & weights replicated):
  Phase A (per core): node tables Ys = x @ W_src.T + b, Yd = x @ W_dest.T,
    stored fp16 as PAIR-ROW tables [N/2, 256] (row k = nodes 2k,2k+1).
  Phase B (per core), per 2048-edge supertile:
    - one dma_gather per endpoint table (int16 pair indices = node>>1,
      2048 idx/call) fetches both nodes of each pair (512B rows)
    - DVE parity select picks the right half per edge (host parity masks)
    - z = edge_attr @ W_edge.T on PE (edge_attr passed host-transposed)
    - out = sel_src + sel_dst + z, batched store
"""

import sys
import numpy as np

if "/opt/trn_rl_repo" not in sys.path:
    sys.path.insert(0, "/opt/trn_rl_repo")

P = 128
CHUNK_IDX = 1024   # indices per dma_gather call (HW descriptor-ring limit)

N_CORES = 8
N_NODES = 50000
N_IN_NODE = 128
N_IN_EDGE = 64
N_OUT = 128
N_EDGES = 1000000
E_CORE = N_EDGES // N_CORES          # 125000
K_SUP = 16                           # 128-edge tiles per supertile
T_TILES = ((E_CORE + P - 1) // P + K_SUP - 1) // K_SUP * K_SUP   # 992
E_PAD = T_TILES * P                  # 126976
S_SUP = T_TILES // K_SUP             # 62
NODES_PAD = (N_NODES + 255) // 256 * 256   # 50176 (pair rows: 25088)
A_TILES = NODES_PAD // P             # 392


def build_program(
    n_cores=N_CORES,
    nodes_pad=NODES_PAD,
    e_pad=E_PAD,
    k_sup=K_SUP,
):
    """Build the Bass program. Returns the compiled Bacc object."""
    import concourse.mybir as mybir
    import concourse.tile as tile
    from concourse import bacc
    from concourse import bass as cbass

    f32 = mybir.dt.float32
    f16 = mybir.dt.float16
    i16 = mybir.dt.int16

    a_tiles = nodes_pad // P
    t_tiles = e_pad // P
    s_sup = t_tiles // k_sup
    n_idx = k_sup * P                       # indices per dma_gather call
    idx_cols = n_idx // 16                  # int16 idx columns per supertile
    d_comb = 2 * N_OUT                      # 256
    pair_rows = nodes_pad // 2

    nc = bacc.Bacc("TRN2", target_bir_lowering=False, debug=False,
                   num_devices=n_cores)

    x_d = nc.dram_tensor("x", [nodes_pad, N_IN_NODE], f16, kind="ExternalInput").ap()
    wct_d = nc.dram_tensor("wct", [N_IN_NODE, d_comb], f16, kind="ExternalInput").ap()
    wet_d = nc.dram_tensor("wet", [N_IN_EDGE, N_OUT], f32, kind="ExternalInput").ap()
    bias_d = nc.dram_tensor("bias", [P, 2 * d_comb], f32, kind="ExternalInput").ap()
    gs_d = nc.dram_tensor("gs", [P, s_sup * idx_cols], i16, kind="ExternalInput").ap()
    gd_d = nc.dram_tensor("gd", [P, s_sup * idx_cols], i16, kind="ExternalInput").ap()
    ps_d = nc.dram_tensor("ps", [P, 2 * t_tiles], f16, kind="ExternalInput").ap()
    pd_d = nc.dram_tensor("pd", [P, 2 * t_tiles], f16, kind="ExternalInput").ap()
    eat_d = nc.dram_tensor("eat", [N_IN_EDGE, e_pad], f32, kind="ExternalInput").ap()
    out_d = nc.dram_tensor("out", [e_pad, N_OUT], f32, kind="ExternalOutput").ap()
    ys_d = nc.dram_tensor("ys", [pair_rows, d_comb], f16, kind="Internal").ap()
    yd_d = nc.dram_tensor("yd", [pair_rows, d_comb], f16, kind="Internal").ap()
    # node-row views of the pair tables for phase A stores
    ys_v = ys_d.rearrange("k (j f) -> (k j) f", j=2)
    yd_v = yd_d.rearrange("k (j f) -> (k j) f", j=2)

    GRP = 8  # node tiles per phase-A group

    with tile.TileContext(nc) as tc:
        with tc.tile_pool(name="static", bufs=1) as spool:
            wct_sb = spool.tile([N_IN_NODE, d_comb], f16)
            nc.sync.dma_start(wct_sb[:], wct_d[:, :])
            wet_sb = spool.tile([N_IN_EDGE, N_OUT], f32)
            nc.sync.dma_start(wet_sb[:], wet_d[:, :])
            bias_sb = spool.tile([P, 2 * d_comb], f32)
            nc.sync.dma_start(bias_sb[:], bias_d[:, :])
            gs_sb = spool.tile([P, s_sup * idx_cols], i16)
            nc.sync.dma_start(gs_sb[:], gs_d[:, :])
            gd_sb = spool.tile([P, s_sup * idx_cols], i16)
            nc.sync.dma_start(gd_sb[:], gd_d[:, :])
            ps_sb = spool.tile([P, 2 * t_tiles], f16)
            nc.sync.dma_start(ps_sb[:], ps_d[:, :])
            pd_sb = spool.tile([P, 2 * t_tiles], f16)
            nc.sync.dma_start(pd_sb[:], pd_d[:, :])

            # ---- Phase A: Ys = x @ Wsrc.T + b, Yd = x @ Wdest.T (fp16) ----
            with tc.tile_pool(name="a_sbuf", bufs=3) as apool, \
                 tc.tile_pool(name="a_ps_yc", bufs=4, space="PSUM") as aps_yc:
                for g0 in range(0, a_tiles, GRP):
                    gn = min(GRP, a_tiles - g0)
                    xt_sb = apool.tile([P, GRP * P], f16, tag="xt_sb")
                    nc.sync.dma_start(
                        xt_sb[:, :gn * P],
                        x_d[g0 * P:(g0 + gn) * P, :], transpose=True)
                    yc_sb = apool.tile([P, GRP * d_comb], f16, tag="yc_sb")
                    for h0 in range(0, gn, 2):
                        hn = min(2, gn - h0)
                        yc_ps = aps_yc.tile([P, 2 * d_comb], f32, tag="yc_ps")
                        for i in range(h0, h0 + hn):
                            nc.tensor.matmul(
                                yc_ps[:, (i - h0) * d_comb:(i - h0 + 1) * d_comb],
                                lhsT=xt_sb[:, i * P:(i + 1) * P],
                                rhs=wct_sb[:], start=True, stop=True)
                        nc.vector.tensor_add(
                            yc_sb[:, h0 * d_comb:(h0 + hn) * d_comb],
                            yc_ps[:, :hn * d_comb],
                            bias_sb[:, :hn * d_comb])
                    # batched stores: ys rows g0*P..(g0+gn)*P from strided cols
                    yc_v = yc_sb.rearrange("p (g c) -> p g c", c=d_comb)
                    ys_rows = ys_v[g0 * P:(g0 + gn) * P, :].rearrange(
                        "(g p) f -> p g f", p=P)
                    yd_rows = yd_v[g0 * P:(g0 + gn) * P, :].rearrange(
                        "(g p) f -> p g f", p=P)
                    nc.sync.dma_start(ys_rows[:, :, :], yc_v[:, :gn, 0:N_OUT])
                    nc.sync.dma_start(yd_rows[:, :, :], yc_v[:, :gn, N_OUT:d_comb])

            tc.strict_bb_all_engine_barrier()

            # ---- Phase B ----
            out_v = out_d.rearrange("(t p) o -> p t o", p=P)
            sup_cols = k_sup * P
            with tc.tile_pool(name="b_sbuf", bufs=2) as bpool, \
                 tc.tile_pool(name="b_psum", bufs=4, space="PSUM") as bpsum:
                for s in range(s_sup):
                    j0 = s * k_sup
                    # 512-idx chunks: larger single dma_gather calls overflow
                    # the SWDGE descriptor ring and hang the device
                    ch_idx = min(CHUNK_IDX, n_idx)
                    ch_tiles = ch_idx // P
                    ch_cols = ch_idx // 16
                    n_ch = n_idx // ch_idx
                    gsrc = bpool.tile([P, k_sup, d_comb], f16, tag="gsrc")
                    gdst = bpool.tile([P, k_sup, d_comb], f16, tag="gdst")
                    for c in range(n_ch):
                        c0 = s * idx_cols + c * ch_cols
                        nc.gpsimd.dma_gather(
                            out_ap=gsrc[:, c * ch_tiles:(c + 1) * ch_tiles, :],
                            in_ap=ys_d[:, :],
                            idxs_ap=gs_sb[:, c0:c0 + ch_cols],
                            num_idxs=ch_idx, num_idxs_reg=ch_idx,
                            elem_size=d_comb)
                        nc.gpsimd.dma_gather(
                            out_ap=gdst[:, c * ch_tiles:(c + 1) * ch_tiles, :],
                            in_ap=yd_d[:, :],
                            idxs_ap=gd_sb[:, c0:c0 + ch_cols],
                            num_idxs=ch_idx, num_idxs_reg=ch_idx,
                            elem_size=d_comb)
                    eat_sb = bpool.tile([N_IN_EDGE, sup_cols], f32, tag="eat_sb")
                    nc.sync.dma_start(
                        eat_sb[:], eat_d[:, j0 * P:(j0 + k_sup) * P])

                    # parity select: res = lo + par*(hi-lo), per endpoint
                    par_s = ps_sb[:, 2 * j0:2 * (j0 + k_sup)].rearrange(
                        "p (g two) -> p g two", two=2)
                    par_d = pd_sb[:, 2 * j0:2 * (j0 + k_sup)].rearrange(
                        "p (g two) -> p g two", two=2)
                    us = bpool.tile([P, k_sup, N_OUT], f16, tag="us")
                    nc.vector.tensor_sub(
                        us[:, :, :], gsrc[:, :, N_OUT:d_comb], gsrc[:, :, 0:N_OUT])
                    nc.vector.tensor_mul(
                        us[:, :, :], us[:, :, :],
                        par_s[:, :, 0:1].to_broadcast([P, k_sup, N_OUT]))
                    ud = bpool.tile([P, k_sup, N_OUT], f16, tag="ud")
                    nc.vector.tensor_sub(
                        ud[:, :, :], gdst[:, :, N_OUT:d_comb], gdst[:, :, 0:N_OUT])
                    nc.vector.tensor_mul(
                        ud[:, :, :], ud[:, :, :],
                        par_d[:, :, 0:1].to_broadcast([P, k_sup, N_OUT]))
                    q = bpool.tile([P, k_sup, N_OUT], f32, tag="q")
                    nc.vector.tensor_add(
                        q[:, :, :], gsrc[:, :, 0:N_OUT], gdst[:, :, 0:N_OUT])
                    tsum = bpool.tile([P, k_sup, N_OUT], f32, tag="tsum")
                    nc.vector.tensor_add(tsum[:, :, :], us[:, :, :], ud[:, :, :])
                    nc.vector.tensor_add(tsum[:, :, :], tsum[:, :, :], q[:, :, :])

                    outsb = bpool.tile([P, sup_cols], f32, tag="outsb")
                    tsum_f = tsum.rearrange("p g o -> p (g o)")
                    for bank in range(k_sup // 4):
                        z_ps = bpsum.tile([P, 4 * P], f32, tag="z_ps")
                        for jj in range(4):
                            t_loc = bank * 4 + jj
                            nc.tensor.matmul(
                                z_ps[:, jj * P:(jj + 1) * P],
                                lhsT=eat_sb[:, t_loc * P:(t_loc + 1) * P],
                                rhs=wet_sb[:], start=True, stop=True)
                        nc.vector.tensor_add(
                            outsb[:, bank * 4 * P:(bank + 1) * 4 * P],
                            z_ps[:], tsum_f[:, bank * 4 * P:(bank + 1) * 4 * P])
                    nc.sync.dma_start(out_v[:, j0:j0 + k_sup, :], outsb[:])

    nc.compile()
    return nc


def _idx_wrap16(seq_i16, n_idx):
    """Pack a flat int16 index sequence into the dma_gather SBUF layout:
    index i at (partition i%16, column i//16), replicated to 8x16 rows."""
    cols = n_idx // 16
    blocks = seq_i16.reshape(-1, cols, 16)           # [S, cols, 16]
    arr = blocks.transpose(0, 2, 1).reshape(-1, 16, cols)  # [S, 16, cols]
    out = np.concatenate([np.tile(a, (8, 1)) for a in arr], axis=1)
    return np.ascontiguousarray(out)                 # [128, S*cols]


def prep_inputs(x, edge_index, edge_attr, W, b,
                n_cores=N_CORES, e_pad=E_PAD, nodes_pad=NODES_PAD,
                k_sup=K_SUP):
    """Host-side input prep: shard + pad + layout. Returns list of in_maps."""
    x = np.asarray(x, dtype=np.float32)
    edge_index = np.asarray(edge_index)
    edge_attr = np.asarray(edge_attr, dtype=np.float32)
    W = np.asarray(W, dtype=np.float32)
    b = np.asarray(b, dtype=np.float32)

    n_nodes, d_node = x.shape
    e_total = edge_index.shape[1]
    e_core = e_total // n_cores
    d_out = W.shape[0]
    d_edge = edge_attr.shape[1]
    t_tiles = e_pad // P
    n_idx = k_sup * P

    x_pad = np.zeros((nodes_pad, d_node), dtype=np.float16)
    x_pad[:n_nodes] = x.astype(np.float16)
    wct = np.ascontiguousarray(np.concatenate(
        [W[:, :d_node].T, W[:, d_node:2 * d_node].T], axis=1)).astype(np.float16)
    wet = np.ascontiguousarray(W[:, 2 * d_node:].T)
    bias_comb = np.concatenate(
        [np.tile(b, (P, 1)), np.zeros((P, d_out), dtype=np.float32)], axis=1)
    bias_full = np.ascontiguousarray(
        np.tile(bias_comb, (1, 2)).astype(np.float32))

    src = np.ascontiguousarray(edge_index[0]).astype(np.int32)
    dst = np.ascontiguousarray(edge_index[1]).astype(np.int32)

    in_maps = []
    for c in range(n_cores):
        lo, hi = c * e_core, (c + 1) * e_core
        src_pad = np.zeros(e_pad, dtype=np.int32)
        src_pad[:e_core] = src[lo:hi]
        dst_pad = np.zeros(e_pad, dtype=np.int32)
        dst_pad[:e_core] = dst[lo:hi]
        chunk = min(CHUNK_IDX, n_idx)
        gs = _idx_wrap16((src_pad >> 1).astype(np.int16), chunk)
        gd = _idx_wrap16((dst_pad >> 1).astype(np.int16), chunk)
        # parity masks in t-major tile layout, duplicated (mask, 0) pairs so
        # device can broadcast-slice [:, :, 0:1]
        ps = np.zeros((P, 2 * t_tiles), dtype=np.float16)
        ps[:, 0::2] = (src_pad & 1).astype(np.float16).reshape(t_tiles, P).T
        pd = np.zeros((P, 2 * t_tiles), dtype=np.float16)
        pd[:, 0::2] = (dst_pad & 1).astype(np.float16).reshape(t_tiles, P).T
        ea_pad = np.zeros((e_pad, d_edge), dtype=np.float32)
        ea_pad[:e_core] = edge_attr[lo:hi]
        eat = np.ascontiguousarray(ea_pad.T)
        in_maps.append({
            "x": x_pad, "wct": wct, "wet": wet, "bias": bias_full,
            "gs": gs, "gd": gd, "ps": ps, "pd": pd, "eat": eat,
        })
    return in_maps


_NC_CACHE = {}


def _get_program():
    key = "full"
    if key not in _NC_CACHE:
        _NC_CACHE[key] = build_program()
    return _NC_CACHE[key]


def run_on_hw(in_maps, nc=None, trace=False, n_cores=N_CORES):
    from concourse import bass_utils
    if nc is None:
        nc = _get_program()
    kw = {}
    if trace:
        _install_profile_hook(bass_utils)
        kw["trace"] = True
    res = bass_utils.run_bass_kernel_spmd(
        nc, in_maps, core_ids=list(range(n_cores)), **kw)
    return res


def _install_profile_hook(bass_utils):
    """Inject the NTFF profile hook missing from this image's antenv."""
    import types
    if "antenv.axon_hooks" in sys.modules:
        return
    try:
        from trn_agent_boot.trn_boot import _ntff_profile_via_ctypes
        hook = _ntff_profile_via_ctypes("/opt/axon/libaxon_pjrt.so")
    except Exception:
        hook = None
    mod = types.ModuleType("antenv.axon_hooks")
    mod.get_axon_ntff_profile_hook = lambda: hook
    mod.set_axon_ntff_profile_hook = lambda h: None
    sys.modules["antenv.axon_hooks"] = mod
    bass_utils.upload_artifacts = lambda tmpdir: f"file://{tmpdir}"


def kernel(x, edge_index, edge_attr, W, b):
    in_maps = prep_inputs(x, edge_index, edge_attr, W, b)
    res = run_on_hw(in_maps)
    e_core = edge_index.shape[1] // N_CORES
    outs = [res.results[c]["out"][:e_core] for c in range(N_CORES)]
    return np.concatenate(outs, axis=0)



# revision 21
# speedup vs baseline: 2.2252x; 2.2252x over previous
"""EdgeOnlyConv GNN message-passing kernel for Trainium2 (8 NeuronCores).

out[e] = concat(x[src[e]], x[dest[e]], edge_attr[e]) @ W.T + b
       = Ys[src[e]] + Yd[dest[e]] + edge_attr[e] @ We.T        (Ys folds bias)

Gather-free edge-parallel design (v2).  dma_gather descriptor generation on
the Q7 SWDGE path costs ~8.6 ns/descriptor (2 of 8 GpSimd cores), which
capped the previous kernel at ~2.2 ms for 250k descriptors/core.  This
version never generates per-edge descriptors:

  Host: per core and per endpoint, sort edges by node id; greedily cut the
    sorted stream into <=512-edge chunks whose node-id span fits a 256-row
    slab; emit per-chunk slab bases, per-edge slab-local indices (int8,
    biased by -128), endpoint-permuted edge_attr, and column->edge maps.
  Device phase A: node tables Ys = x@Wsrc.T + b, Yd = x@Wdst.T (fp16,
    node-major) built on PE in one pass over xT, stored to DRAM.
  Device per chunk (both passes interleaved):
    - dynamic-offset DMA stages the 256-row slab [128p, 2slot, 128f]
    - gpsimd.partition_broadcast replicates the int8 local-idx row
    - DVE is_equal vs per-partition constants -> two one-hot f16 tiles
    - PE: psum[f,e] = slab0.T@E0 + slab1.T@E1 (+ We.T@edge_attr on pass A)
    - ACT copies psum -> fp16 staging, feature-major DMA store
  Host: un-permute the two partial outputs (f32) and add.
"""

import sys
import numpy as np

if "/opt/trn_rl_repo" not in sys.path:
    sys.path.insert(0, "/opt/trn_rl_repo")

P = 128
CH = 512          # edge columns per chunk (PSUM bank = 512 f32)
SLAB = 256        # node rows staged per chunk (2 matmul slots)
N_CORES = 8
N_NODES = 50000
N_IN_NODE = 128
N_IN_EDGE = 64
N_OUT = 128
N_EDGES = 1000000
E_CORE = N_EDGES // N_CORES            # 125000
NODES_PAD = (N_NODES + P - 1) // P * P  # 50176
A_TILES = NODES_PAD // P               # 392


def build_program(n_ch, nodes_pad=NODES_PAD, n_cores=N_CORES):
    """n_ch: chunks per pass (same for all cores; host pads to this)."""
    import concourse.mybir as mybir
    import concourse.tile as tile
    from concourse import bacc
    from concourse import bass as cbass

    f32 = mybir.dt.float32
    f16 = mybir.dt.float16
    i8 = mybir.dt.int8
    i32 = mybir.dt.int32
    EQ = mybir.AluOpType.is_equal
    ds = cbass.ds

    e_dev = n_ch * CH
    a_tiles = nodes_pad // P

    nc = bacc.Bacc("TRN2", target_bir_lowering=False, debug=False,
                   num_devices=n_cores)

    xT_d = nc.dram_tensor("xT", [P, nodes_pad], f16, kind="ExternalInput").ap()
    wsT_d = nc.dram_tensor("wsT", [P, P], f16, kind="ExternalInput").ap()
    wdT_d = nc.dram_tensor("wdT", [P, P], f16, kind="ExternalInput").ap()
    weT_d = nc.dram_tensor("weT", [N_IN_EDGE, P], f16, kind="ExternalInput").ap()
    bias_d = nc.dram_tensor("bias", [P, P], f32, kind="ExternalInput").ap()
    pidx_d = nc.dram_tensor("pidx", [P, 2], i8, kind="ExternalInput").ap()
    basA_d = nc.dram_tensor("basA", [1, n_ch], i32, kind="ExternalInput").ap()
    basB_d = nc.dram_tensor("basB", [1, n_ch], i32, kind="ExternalInput").ap()
    liA_d = nc.dram_tensor("liA", [1, e_dev], i8, kind="ExternalInput").ap()
    liB_d = nc.dram_tensor("liB", [1, e_dev], i8, kind="ExternalInput").ap()
    eatA_d = nc.dram_tensor("eatA", [N_IN_EDGE, e_dev], f16, kind="ExternalInput").ap()
    outA_d = nc.dram_tensor("outA", [P, e_dev], f16, kind="ExternalOutput").ap()
    outB_d = nc.dram_tensor("outB", [P, e_dev], f16, kind="ExternalOutput").ap()
    ys_d = nc.dram_tensor("ys", [nodes_pad, P], f16, kind="Internal").ap()
    yd_d = nc.dram_tensor("yd", [nodes_pad, P], f16, kind="Internal").ap()

    GRP = 16   # node tiles per phase-A group
    LGRP = 16  # chunks per li-row load
    OGRP = 4   # chunks per output staging group

    with tile.TileContext(nc) as tc:
        with tc.tile_pool(name="static", bufs=1) as spool:
            wsT_sb = spool.tile([P, P], f16)
            nc.sync.dma_start(wsT_sb[:], wsT_d[:, :])
            wdT_sb = spool.tile([P, P], f16)
            nc.sync.dma_start(wdT_sb[:], wdT_d[:, :])
            weT_sb = spool.tile([N_IN_EDGE, P], f16)
            nc.sync.dma_start(weT_sb[:], weT_d[:, :])
            bias_sb = spool.tile([P, P], f32)
            nc.sync.dma_start(bias_sb[:], bias_d[:, :])
            pidx_sb = spool.tile([P, 2], i8)
            nc.sync.dma_start(pidx_sb[:], pidx_d[:, :])
            basA_sb = spool.tile([1, n_ch], i32)
            nc.sync.dma_start(basA_sb[:], basA_d[:, :])
            basB_sb = spool.tile([1, n_ch], i32)
            nc.sync.dma_start(basB_sb[:], basB_d[:, :])

            # ---- Phase A: Ys = x@Wsrc.T + b, Yd = x@Wdst.T (node-major) ----
            with tc.tile_pool(name="pa", bufs=2) as papool, \
                 tc.tile_pool(name="paps", bufs=4, space="PSUM") as paps:
                for g0 in range(0, a_tiles, GRP):
                    gn = min(GRP, a_tiles - g0)
                    xt = papool.tile([P, GRP * P], f16, tag="xt")
                    nc.sync.dma_start(xt[:, :gn * P],
                                      xT_d[:, g0 * P:(g0 + gn) * P])
                    ysb = papool.tile([P, GRP, P], f16, tag="ysb")
                    ydb = papool.tile([P, GRP, P], f16, tag="ydb")
                    for t in range(gn):
                        ps = paps.tile([P, 2 * P], f32, tag="ps")
                        nc.tensor.matmul(ps[:, 0:P],
                                         lhsT=xt[:, t * P:(t + 1) * P],
                                         rhs=wsT_sb[:], start=True, stop=True)
                        nc.tensor.matmul(ps[:, P:2 * P],
                                         lhsT=xt[:, t * P:(t + 1) * P],
                                         rhs=wdT_sb[:], start=True, stop=True)
                        nc.vector.tensor_add(ysb[:, t, :], ps[:, 0:P], bias_sb[:])
                        nc.scalar.copy(ydb[:, t, :], ps[:, P:2 * P])
                    ys_rows = ys_d[g0 * P:(g0 + gn) * P, :].rearrange(
                        "(t p) f -> p t f", p=P)
                    yd_rows = yd_d[g0 * P:(g0 + gn) * P, :].rearrange(
                        "(t p) f -> p t f", p=P)
                    nc.sync.dma_start(ys_rows[:, :, :], ysb[:, :gn, :])
                    nc.scalar.dma_start(yd_rows[:, :, :], ydb[:, :gn, :])

            tc.strict_bb_all_engine_barrier()

            # ---- Passes A (src) and B (dst), interleaved chunk loop ----
            with tc.tile_pool(name="pb", bufs=3) as bpool, \
                 tc.tile_pool(name="bps", bufs=4, space="PSUM") as bps:
                passes = [
                    ("A", basA_sb, liA_d, ys_d, outA_d, True),
                    ("B", basB_sb, liB_d, yd_d, outB_d, False),
                ]
                li_rows = {}
                eat_rows = {}
                outst = {}
                dma_engs = [nc.sync, nc.scalar]
                # cycling register pools: fresh registers per value_load
                # otherwise exhaust the 54 allocatable GPRs per sequencer
                slab_regs = [
                    [e.alloc_register(f"slabbase{i}_{j}") for j in range(3)]
                    for i, e in enumerate(dma_engs)
                ]
                reg_cnt = [0, 0]
                for k in range(n_ch):
                    for (tagp, bas_sb, li_d, tab_d, out_d, has_z) in passes:
                        if k % LGRP == 0:
                            lw = min(LGRP, n_ch - k) * CH
                            lr = bpool.tile([1, LGRP * CH], i8, tag=f"li{tagp}")
                            nc.scalar.dma_start(
                                lr[:, :lw], li_d[0:1, k * CH:k * CH + lw])
                            li_rows[tagp] = lr
                            if has_z:
                                er = bpool.tile([N_IN_EDGE, LGRP * CH], f16,
                                                tag="eat")
                                nc.sync.dma_start(
                                    er[:, :lw],
                                    eatA_d[:, k * CH:k * CH + lw])
                                eat_rows[tagp] = er
                        ei = k % 2
                        eng = dma_engs[ei]
                        r = slab_regs[ei][reg_cnt[ei] % 3]
                        reg_cnt[ei] += 1
                        eng.reg_load(r, bas_sb[0:1, k:k + 1])
                        base = eng.snap(r, min_val=0,
                                        max_val=nodes_pad - SLAB)
                        slab = bpool.tile([P, 2, P], f16, tag=f"slab{tagp}")
                        eng.dma_start(
                            slab[:, :, :],
                            tab_d[ds(base, SLAB), :].rearrange(
                                "(s p) f -> p s f", p=P))
                        libc = bpool.tile([P, CH], i8, tag=f"libc{tagp}")
                        nc.gpsimd.partition_broadcast(
                            libc[:, :],
                            li_rows[tagp][0:1, (k % LGRP) * CH:(k % LGRP + 1) * CH])
                        eT0 = bpool.tile([P, CH], f16, tag=f"eT0{tagp}")
                        nc.vector.tensor_tensor(
                            eT0[:, :], libc[:, :],
                            pidx_sb[:, 0:1].to_broadcast([P, CH]), op=EQ)
                        eT1 = bpool.tile([P, CH], f16, tag=f"eT1{tagp}")
                        nc.vector.tensor_tensor(
                            eT1[:, :], libc[:, :],
                            pidx_sb[:, 1:2].to_broadcast([P, CH]), op=EQ)
                        po = bps.tile([P, CH], f32, tag=f"po{tagp}")
                        nc.tensor.matmul(po[:, :], lhsT=slab[:, 0, :],
                                         rhs=eT0[:, :], start=True, stop=False)
                        nc.tensor.matmul(po[:, :], lhsT=slab[:, 1, :],
                                         rhs=eT1[:, :], start=False,
                                         stop=not has_z)
                        if has_z:
                            nc.tensor.matmul(
                                po[:, :], lhsT=weT_sb[:],
                                rhs=eat_rows[tagp][
                                    :, (k % LGRP) * CH:(k % LGRP + 1) * CH],
                                start=False, stop=True)
                        if k % OGRP == 0:
                            ot = bpool.tile([P, OGRP * CH], f16,
                                            tag=f"outst{tagp}",
                                            name=f"outst{tagp}_{k}")
                            outst[tagp] = ot
                        nc.scalar.copy(
                            outst[tagp][:, (k % OGRP) * CH:(k % OGRP + 1) * CH],
                            po[:, :])
                        if k % OGRP == OGRP - 1 or k == n_ch - 1:
                            k0 = k - (k % OGRP)
                            dma_engs[(k + 1) % 2].dma_start(
                                out_d[:, k0 * CH:(k + 1) * CH],
                                outst[tagp][:, :(k % OGRP + 1) * CH])

    nc.compile()
    return nc


def _plan_pass(key_sorted):
    """Greedy chunking of a sorted node-id stream.

    Returns list of (start, count, base): count <= CH edges starting at
    `start` whose ids fit in [base, base+SLAB)."""
    n = len(key_sorted)
    chunks = []
    i = 0
    while i < n:
        base = int(key_sorted[i])
        base = min(base, NODES_PAD - SLAB)
        j = min(i + CH, n)
        # first index whose id falls outside the slab
        j = i + int(np.searchsorted(key_sorted[i:j], base + SLAB, side="left"))
        chunks.append((i, j - i, base))
        i = j
    return chunks


def prep_core(src, dst, edge_attr_core):
    """Per-core host prep. Returns dict of device arrays + colmaps."""
    plans = {}
    for tagp, key in (("A", src), ("B", dst)):
        perm = np.argsort(key, kind="stable")
        ks = key[perm].astype(np.int64)
        chunks = _plan_pass(ks)
        plans[tagp] = (perm, ks, chunks)
    return plans


def pack_core(plans, edge_attr_core, n_ch):
    e_dev = n_ch * CH
    dev = {}
    colmaps = {}
    for tagp, (perm, ks, chunks) in plans.items():
        li = np.zeros(e_dev, dtype=np.int8)
        bases = np.zeros(n_ch, dtype=np.int32)
        colmap = np.full(e_dev, -1, dtype=np.int64)
        for c, (s, cnt, base) in enumerate(chunks):
            bases[c] = base
            sl = slice(c * CH, c * CH + cnt)
            li[sl] = (ks[s:s + cnt] - base - 128).astype(np.int8)
            colmap[sl] = perm[s:s + cnt]
        dev[f"li{tagp}"] = li[None, :]
        dev[f"bas{tagp}"] = bases[None, :]
        colmaps[tagp] = colmap
        if tagp == "A":
            eat = np.zeros((N_IN_EDGE, e_dev), dtype=np.float16)
            valid = colmap >= 0
            eat[:, valid] = edge_attr_core[colmap[valid]].astype(np.float16).T
            dev["eatA"] = eat
    return dev, colmaps


def prep_inputs(x, edge_index, edge_attr, W, b):
    """Host-side prep: shard + sort + pack. Returns (in_maps, colmaps, n_ch)."""
    x = np.asarray(x, dtype=np.float32)
    edge_index = np.asarray(edge_index)
    edge_attr = np.asarray(edge_attr, dtype=np.float32)
    W = np.asarray(W, dtype=np.float32)
    b = np.asarray(b, dtype=np.float32)

    xT = np.zeros((P, NODES_PAD), dtype=np.float16)
    xT[:, :N_NODES] = x.astype(np.float16).T
    wsT = np.ascontiguousarray(W[:, :P].T).astype(np.float16)
    wdT = np.ascontiguousarray(W[:, P:2 * P].T).astype(np.float16)
    weT = np.ascontiguousarray(W[:, 2 * P:].T).astype(np.float16)
    bias_rep = np.ascontiguousarray(
        np.tile(b[None, :].astype(np.float32), (P, 1)))
    pidx = np.stack([np.arange(P) - 128, np.arange(P)],
                    axis=1).astype(np.int8)

    src = np.ascontiguousarray(edge_index[0]).astype(np.int64)
    dst = np.ascontiguousarray(edge_index[1]).astype(np.int64)

    core_plans = []
    n_ch = 0
    for c in range(N_CORES):
        lo, hi = c * E_CORE, (c + 1) * E_CORE
        plans = prep_core(src[lo:hi], dst[lo:hi], None)
        for tagp in ("A", "B"):
            n_ch = max(n_ch, len(plans[tagp][2]))
        core_plans.append(plans)

    in_maps = []
    all_colmaps = []
    for c in range(N_CORES):
        lo, hi = c * E_CORE, (c + 1) * E_CORE
        dev, colmaps = pack_core(core_plans[c], edge_attr[lo:hi], n_ch)
        dev.update({
            "xT": xT, "wsT": wsT, "wdT": wdT, "weT": weT,
            "bias": bias_rep, "pidx": pidx,
        })
        in_maps.append(dev)
        all_colmaps.append(colmaps)
    return in_maps, all_colmaps, n_ch


_NC_CACHE = {}


def _get_program(n_ch):
    if n_ch not in _NC_CACHE:
        _NC_CACHE[n_ch] = build_program(n_ch)
    return _NC_CACHE[n_ch]


def run_on_hw(in_maps, nc=None, trace=False, n_cores=N_CORES):
    from concourse import bass_utils
    if nc is None:
        raise ValueError("pass nc")
    kw = {}
    if trace:
        _install_profile_hook(bass_utils)
        kw["trace"] = True
    res = bass_utils.run_bass_kernel_spmd(
        nc, in_maps, core_ids=list(range(n_cores)), **kw)
    return res


def _install_profile_hook(bass_utils):
    """Inject the NTFF profile hook missing from this image's antenv."""
    import types
    if "antenv.axon_hooks" in sys.modules:
        return
    try:
        from trn_agent_boot.trn_boot import _ntff_profile_via_ctypes
        hook = _ntff_profile_via_ctypes("/opt/axon/libaxon_pjrt.so")
    except Exception:
        hook = None
    mod = types.ModuleType("antenv.axon_hooks")
    mod.get_axon_ntff_profile_hook = lambda: hook
    mod.set_axon_ntff_profile_hook = lambda h: None
    sys.modules["antenv.axon_hooks"] = mod
    bass_utils.upload_artifacts = lambda tmpdir: f"file://{tmpdir}"


def combine_outputs(res, all_colmaps):
    out = np.zeros((N_EDGES, N_OUT), dtype=np.float32)
    for c in range(N_CORES):
        lo = c * E_CORE
        cmA = all_colmaps[c]["A"]
        cmB = all_colmaps[c]["B"]
        outA = np.asarray(res.results[c]["outA"])  # [128, e_dev] f16
        outB = np.asarray(res.results[c]["outB"])
        vA = cmA >= 0
        vB = cmB >= 0
        out[lo + cmA[vA]] = outA[:, vA].T.astype(np.float32)
        out[lo + cmB[vB]] += outB[:, vB].T.astype(np.float32)
    return out


def kernel(x, edge_index, edge_attr, W, b):
    in_maps, all_colmaps, n_ch = prep_inputs(x, edge_index, edge_attr, W, b)
    nc = _get_program(n_ch)
    res = run_on_hw(in_maps, nc=nc)
    return combine_outputs(res, all_colmaps)


# revision 23
# speedup vs baseline: 2.2817x; 1.0254x over previous
"""EdgeOnlyConv GNN message-passing kernel for Trainium2 (8 NeuronCores).

out[e] = concat(x[src[e]], x[dest[e]], edge_attr[e]) @ W.T + b
       = Ys[src[e]] + Yd[dest[e]] + edge_attr[e] @ We.T        (Ys folds bias)

Gather-free edge-parallel design (v2).  dma_gather descriptor generation on
the Q7 SWDGE path costs ~8.6 ns/descriptor (2 of 8 GpSimd cores), which
capped the previous kernel at ~2.2 ms for 250k descriptors/core.  This
version never generates per-edge descriptors:

  Host: per core and per endpoint, sort edges by node id; greedily cut the
    sorted stream into <=512-edge chunks whose node-id span fits a 256-row
    slab; emit per-chunk slab bases, per-edge slab-local indices (int8,
    biased by -128), endpoint-permuted edge_attr, and column->edge maps.
  Device phase A: node tables Ys = x@Wsrc.T + b, Yd = x@Wdst.T (fp16,
    node-major) built on PE in one pass over xT, stored to DRAM.
  Device per chunk (both passes interleaved):
    - dynamic-offset DMA stages the 256-row slab [128p, 2slot, 128f]
    - gpsimd.partition_broadcast replicates the int8 local-idx row
    - DVE is_equal vs per-partition constants -> two one-hot f16 tiles
    - PE: psum[f,e] = slab0.T@E0 + slab1.T@E1 (+ We.T@edge_attr on pass A)
    - ACT copies psum -> fp16 staging, feature-major DMA store
  Host: un-permute the two partial outputs (f32) and add.
"""

import sys
import numpy as np

if "/opt/trn_rl_repo" not in sys.path:
    sys.path.insert(0, "/opt/trn_rl_repo")

P = 128
CH = 512          # edge columns per chunk (PSUM bank = 512 f32)
SLAB = 256        # node rows staged per chunk (2 matmul slots)
N_CORES = 8
N_NODES = 50000
N_IN_NODE = 128
N_IN_EDGE = 64
N_OUT = 128
N_EDGES = 1000000
E_CORE = N_EDGES // N_CORES            # 125000
NODES_PAD = (N_NODES + P - 1) // P * P  # 50176
A_TILES = NODES_PAD // P               # 392


def build_program(n_ch, nodes_pad=NODES_PAD, n_cores=N_CORES):
    """n_ch: chunks per pass (same for all cores; host pads to this)."""
    import concourse.mybir as mybir
    import concourse.tile as tile
    from concourse import bacc
    from concourse import bass as cbass

    f32 = mybir.dt.float32
    f16 = mybir.dt.float16
    i8 = mybir.dt.int8
    i32 = mybir.dt.int32
    EQ = mybir.AluOpType.is_equal
    ds = cbass.ds

    e_dev = n_ch * CH
    a_tiles = nodes_pad // P

    nc = bacc.Bacc("TRN2", target_bir_lowering=False, debug=False,
                   num_devices=n_cores)

    xT_d = nc.dram_tensor("xT", [P, nodes_pad], f16, kind="ExternalInput").ap()
    wsT_d = nc.dram_tensor("wsT", [P, P], f16, kind="ExternalInput").ap()
    wdT_d = nc.dram_tensor("wdT", [P, P], f16, kind="ExternalInput").ap()
    weT_d = nc.dram_tensor("weT", [N_IN_EDGE, P], f16, kind="ExternalInput").ap()
    bias_d = nc.dram_tensor("bias", [P, P], f32, kind="ExternalInput").ap()
    pidx_d = nc.dram_tensor("pidx", [P, 2 * CH], f16, kind="ExternalInput").ap()
    basA_d = nc.dram_tensor("basA", [1, n_ch], i32, kind="ExternalInput").ap()
    basB_d = nc.dram_tensor("basB", [1, n_ch], i32, kind="ExternalInput").ap()
    liA_d = nc.dram_tensor("liA", [1, e_dev], f16, kind="ExternalInput").ap()
    liB_d = nc.dram_tensor("liB", [1, e_dev], f16, kind="ExternalInput").ap()
    eatA_d = nc.dram_tensor("eatA", [N_IN_EDGE, e_dev], f16, kind="ExternalInput").ap()
    outA_d = nc.dram_tensor("outA", [P, e_dev], f16, kind="ExternalOutput").ap()
    outB_d = nc.dram_tensor("outB", [P, e_dev], f16, kind="ExternalOutput").ap()
    ys_d = nc.dram_tensor("ys", [nodes_pad, P], f16, kind="Internal").ap()
    yd_d = nc.dram_tensor("yd", [nodes_pad, P], f16, kind="Internal").ap()

    GRP = 16   # node tiles per phase-A group
    LGRP = 8   # chunks per li-row load
    OGRP = 4   # chunks per output staging group

    with tile.TileContext(nc) as tc:
        with tc.tile_pool(name="static", bufs=1) as spool:
            wsT_sb = spool.tile([P, P], f16)
            nc.sync.dma_start(wsT_sb[:], wsT_d[:, :])
            wdT_sb = spool.tile([P, P], f16)
            nc.sync.dma_start(wdT_sb[:], wdT_d[:, :])
            weT_sb = spool.tile([N_IN_EDGE, P], f16)
            nc.sync.dma_start(weT_sb[:], weT_d[:, :])
            bias_sb = spool.tile([P, P], f32)
            nc.sync.dma_start(bias_sb[:], bias_d[:, :])
            pidx_sb = spool.tile([P, 2 * CH], f16)
            nc.sync.dma_start(pidx_sb[:], pidx_d[:, :])
            basA_sb = spool.tile([1, n_ch], i32)
            nc.sync.dma_start(basA_sb[:], basA_d[:, :])
            basB_sb = spool.tile([1, n_ch], i32)
            nc.sync.dma_start(basB_sb[:], basB_d[:, :])

            # ---- Phase A: Ys = x@Wsrc.T + b, Yd = x@Wdst.T (node-major) ----
            with tc.tile_pool(name="pa", bufs=2) as papool, \
                 tc.tile_pool(name="paps", bufs=4, space="PSUM") as paps:
                for g0 in range(0, a_tiles, GRP):
                    gn = min(GRP, a_tiles - g0)
                    xt = papool.tile([P, GRP * P], f16, tag="xt")
                    nc.sync.dma_start(xt[:, :gn * P],
                                      xT_d[:, g0 * P:(g0 + gn) * P])
                    ysb = papool.tile([P, GRP, P], f16, tag="ysb")
                    ydb = papool.tile([P, GRP, P], f16, tag="ydb")
                    for t in range(gn):
                        ps = paps.tile([P, 2 * P], f32, tag="ps")
                        nc.tensor.matmul(ps[:, 0:P],
                                         lhsT=xt[:, t * P:(t + 1) * P],
                                         rhs=wsT_sb[:], start=True, stop=True)
                        nc.tensor.matmul(ps[:, P:2 * P],
                                         lhsT=xt[:, t * P:(t + 1) * P],
                                         rhs=wdT_sb[:], start=True, stop=True)
                        nc.vector.tensor_add(ysb[:, t, :], ps[:, 0:P], bias_sb[:])
                        nc.scalar.copy(ydb[:, t, :], ps[:, P:2 * P])
                    ys_rows = ys_d[g0 * P:(g0 + gn) * P, :].rearrange(
                        "(t p) f -> p t f", p=P)
                    yd_rows = yd_d[g0 * P:(g0 + gn) * P, :].rearrange(
                        "(t p) f -> p t f", p=P)
                    nc.sync.dma_start(ys_rows[:, :, :], ysb[:, :gn, :])
                    nc.scalar.dma_start(yd_rows[:, :, :], ydb[:, :gn, :])

            tc.strict_bb_all_engine_barrier()

            # ---- Passes A (src) and B (dst), interleaved chunk loop ----
            with tc.tile_pool(name="pb", bufs=3) as bpool, \
                 tc.tile_pool(name="bps", bufs=4, space="PSUM") as bps:
                passes = [
                    ("A", basA_sb, liA_d, ys_d, outA_d, True),
                    ("B", basB_sb, liB_d, yd_d, outB_d, False),
                ]
                li_rows = {}
                eat_rows = {}
                outst = {}
                dma_engs = [nc.sync, nc.scalar]
                # cycling register pools: fresh registers per value_load
                # otherwise exhaust the 54 allocatable GPRs per sequencer
                slab_regs = [
                    [e.alloc_register(f"slabbase{i}_{j}") for j in range(3)]
                    for i, e in enumerate(dma_engs)
                ]
                reg_cnt = [0, 0]
                for k in range(n_ch):
                    for (tagp, bas_sb, li_d, tab_d, out_d, has_z) in passes:
                        if k % LGRP == 0:
                            lw = min(LGRP, n_ch - k) * CH
                            lr = bpool.tile([P, LGRP * CH], f16,
                                            tag=f"li{tagp}")
                            nc.scalar.dma_start(
                                lr[:, :lw],
                                li_d[0:1, k * CH:k * CH + lw].to_broadcast(
                                    [P, lw]))
                            li_rows[tagp] = lr
                            if has_z:
                                er = bpool.tile([N_IN_EDGE, LGRP * CH], f16,
                                                tag="eat")
                                nc.sync.dma_start(
                                    er[:, :lw],
                                    eatA_d[:, k * CH:k * CH + lw])
                                eat_rows[tagp] = er
                        ei = k % 2
                        eng = dma_engs[ei]
                        r = slab_regs[ei][reg_cnt[ei] % 3]
                        reg_cnt[ei] += 1
                        eng.reg_load(r, bas_sb[0:1, k:k + 1])
                        base = eng.snap(r, min_val=0,
                                        max_val=nodes_pad - SLAB)
                        slab = bpool.tile([P, 2, P], f16, tag=f"slab{tagp}")
                        eng.dma_start(
                            slab[:, :, :],
                            tab_d[ds(base, SLAB), :].rearrange(
                                "(s p) f -> p s f", p=P))
                        libc = li_rows[tagp][
                            :, (k % LGRP) * CH:(k % LGRP + 1) * CH]
                        eT0 = bpool.tile([P, CH], f16, tag=f"eT0{tagp}")
                        nc.vector.tensor_tensor(
                            eT0[:, :], libc, pidx_sb[:, 0:CH], op=EQ)
                        eT1 = bpool.tile([P, CH], f16, tag=f"eT1{tagp}")
                        nc.vector.tensor_tensor(
                            eT1[:, :], libc, pidx_sb[:, CH:2 * CH], op=EQ)
                        po = bps.tile([P, CH], f32, tag=f"po{tagp}")
                        nc.tensor.matmul(po[:, :], lhsT=slab[:, 0, :],
                                         rhs=eT0[:, :], start=True, stop=False)
                        nc.tensor.matmul(po[:, :], lhsT=slab[:, 1, :],
                                         rhs=eT1[:, :], start=False,
                                         stop=not has_z)
                        if has_z:
                            nc.tensor.matmul(
                                po[:, :], lhsT=weT_sb[:],
                                rhs=eat_rows[tagp][
                                    :, (k % LGRP) * CH:(k % LGRP + 1) * CH],
                                start=False, stop=True)
                        if k % OGRP == 0:
                            ot = bpool.tile([P, OGRP * CH], f16,
                                            tag=f"outst{tagp}",
                                            name=f"outst{tagp}_{k}")
                            outst[tagp] = ot
                        nc.scalar.copy(
                            outst[tagp][:, (k % OGRP) * CH:(k % OGRP + 1) * CH],
                            po[:, :])
                        if k % OGRP == OGRP - 1 or k == n_ch - 1:
                            k0 = k - (k % OGRP)
                            dma_engs[(k + 1) % 2].dma_start(
                                out_d[:, k0 * CH:(k + 1) * CH],
                                outst[tagp][:, :(k % OGRP + 1) * CH])

    nc.compile()
    return nc


def _plan_pass(key_sorted):
    """Greedy chunking of a sorted node-id stream.

    Returns list of (start, count, base): count <= CH edges starting at
    `start` whose ids fit in [base, base+SLAB)."""
    n = len(key_sorted)
    chunks = []
    i = 0
    while i < n:
        base = int(key_sorted[i])
        base = min(base, NODES_PAD - SLAB)
        j = min(i + CH, n)
        # first index whose id falls outside the slab
        j = i + int(np.searchsorted(key_sorted[i:j], base + SLAB, side="left"))
        chunks.append((i, j - i, base))
        i = j
    return chunks


def prep_core(src, dst, edge_attr_core):
    """Per-core host prep. Returns dict of device arrays + colmaps."""
    plans = {}
    for tagp, key in (("A", src), ("B", dst)):
        perm = np.argsort(key, kind="stable")
        ks = key[perm].astype(np.int64)
        chunks = _plan_pass(ks)
        plans[tagp] = (perm, ks, chunks)
    return plans


def pack_core(plans, edge_attr_core, n_ch):
    e_dev = n_ch * CH
    dev = {}
    colmaps = {}
    for tagp, (perm, ks, chunks) in plans.items():
        li = np.zeros(e_dev, dtype=np.float16)
        bases = np.zeros(n_ch, dtype=np.int32)
        colmap = np.full(e_dev, -1, dtype=np.int64)
        for c, (s, cnt, base) in enumerate(chunks):
            bases[c] = base
            sl = slice(c * CH, c * CH + cnt)
            li[sl] = (ks[s:s + cnt] - base).astype(np.float16)
            colmap[sl] = perm[s:s + cnt]
        dev[f"li{tagp}"] = li[None, :]
        dev[f"bas{tagp}"] = bases[None, :]
        colmaps[tagp] = colmap
        if tagp == "A":
            eat = np.zeros((N_IN_EDGE, e_dev), dtype=np.float16)
            valid = colmap >= 0
            eat[:, valid] = edge_attr_core[colmap[valid]].astype(np.float16).T
            dev["eatA"] = eat
    return dev, colmaps


def prep_inputs(x, edge_index, edge_attr, W, b):
    """Host-side prep: shard + sort + pack. Returns (in_maps, colmaps, n_ch)."""
    x = np.asarray(x, dtype=np.float32)
    edge_index = np.asarray(edge_index)
    edge_attr = np.asarray(edge_attr, dtype=np.float32)
    W = np.asarray(W, dtype=np.float32)
    b = np.asarray(b, dtype=np.float32)

    xT = np.zeros((P, NODES_PAD), dtype=np.float16)
    xT[:, :N_NODES] = x.astype(np.float16).T
    wsT = np.ascontiguousarray(W[:, :P].T).astype(np.float16)
    wdT = np.ascontiguousarray(W[:, P:2 * P].T).astype(np.float16)
    weT = np.ascontiguousarray(W[:, 2 * P:].T).astype(np.float16)
    bias_rep = np.ascontiguousarray(
        np.tile(b[None, :].astype(np.float32), (P, 1)))
    pcol = np.arange(P, dtype=np.float16)[:, None]
    pidx = np.concatenate([np.tile(pcol, (1, CH)),
                           np.tile(pcol + 128, (1, CH))],
                          axis=1).astype(np.float16)

    src = np.ascontiguousarray(edge_index[0]).astype(np.int64)
    dst = np.ascontiguousarray(edge_index[1]).astype(np.int64)

    core_plans = []
    n_ch = 0
    for c in range(N_CORES):
        lo, hi = c * E_CORE, (c + 1) * E_CORE
        plans = prep_core(src[lo:hi], dst[lo:hi], None)
        for tagp in ("A", "B"):
            n_ch = max(n_ch, len(plans[tagp][2]))
        core_plans.append(plans)

    in_maps = []
    all_colmaps = []
    for c in range(N_CORES):
        lo, hi = c * E_CORE, (c + 1) * E_CORE
        dev, colmaps = pack_core(core_plans[c], edge_attr[lo:hi], n_ch)
        dev.update({
            "xT": xT, "wsT": wsT, "wdT": wdT, "weT": weT,
            "bias": bias_rep, "pidx": pidx,
        })
        in_maps.append(dev)
        all_colmaps.append(colmaps)
    return in_maps, all_colmaps, n_ch


_NC_CACHE = {}


def _get_program(n_ch):
    if n_ch not in _NC_CACHE:
        _NC_CACHE[n_ch] = build_program(n_ch)
    return _NC_CACHE[n_ch]


def run_on_hw(in_maps, nc=None, trace=False, n_cores=N_CORES):
    from concourse import bass_utils
    if nc is None:
        raise ValueError("pass nc")
    kw = {}
    if trace:
        _install_profile_hook(bass_utils)
        kw["trace"] = True
    res = bass_utils.run_bass_kernel_spmd(
        nc, in_maps, core_ids=list(range(n_cores)), **kw)
    return res


def _install_profile_hook(bass_utils):
    """Inject the NTFF profile hook missing from this image's antenv."""
    import types
    if "antenv.axon_hooks" in sys.modules:
        return
    try:
        from trn_agent_boot.trn_boot import _ntff_profile_via_ctypes
        hook = _ntff_profile_via_ctypes("/opt/axon/libaxon_pjrt.so")
    except Exception:
        hook = None
    mod = types.ModuleType("antenv.axon_hooks")
    mod.get_axon_ntff_profile_hook = lambda: hook
    mod.set_axon_ntff_profile_hook = lambda h: None
    sys.modules["antenv.axon_hooks"] = mod
    bass_utils.upload_artifacts = lambda tmpdir: f"file://{tmpdir}"


def combine_outputs(res, all_colmaps):
    out = np.zeros((N_EDGES, N_OUT), dtype=np.float32)
    for c in range(N_CORES):
        lo = c * E_CORE
        cmA = all_colmaps[c]["A"]
        cmB = all_colmaps[c]["B"]
        outA = np.asarray(res.results[c]["outA"])  # [128, e_dev] f16
        outB = np.asarray(res.results[c]["outB"])
        vA = cmA >= 0
        vB = cmB >= 0
        out[lo + cmA[vA]] = outA[:, vA].T.astype(np.float32)
        out[lo + cmB[vB]] += outB[:, vB].T.astype(np.float32)
    return out


def kernel(x, edge_index, edge_attr, W, b):
    in_maps, all_colmaps, n_ch = prep_inputs(x, edge_index, edge_attr, W, b)
    nc = _get_program(n_ch)
    res = run_on_hw(in_maps, nc=nc)
    return combine_outputs(res, all_colmaps)


# revision 24
# speedup vs baseline: 2.5541x; 1.1194x over previous
"""EdgeOnlyConv GNN message-passing kernel for Trainium2 (8 NeuronCores).

out[e] = concat(x[src[e]], x[dest[e]], edge_attr[e]) @ W.T + b
       = Ys[src[e]] + Yd[dest[e]] + edge_attr[e] @ We.T        (Ys folds bias)

Gather-free edge-parallel design (v2).  dma_gather descriptor generation on
the Q7 SWDGE path costs ~8.6 ns/descriptor (2 of 8 GpSimd cores), which
capped the previous kernel at ~2.2 ms for 250k descriptors/core.  This
version never generates per-edge descriptors:

  Host: per core and per endpoint, sort edges by node id; greedily cut the
    sorted stream into <=512-edge chunks whose node-id span fits a 256-row
    slab; emit per-chunk slab bases, per-edge slab-local indices (int8,
    biased by -128), endpoint-permuted edge_attr, and column->edge maps.
  Device phase A: node tables Ys = x@Wsrc.T + b, Yd = x@Wdst.T (fp16,
    node-major) built on PE in one pass over xT, stored to DRAM.
  Device per chunk (both passes interleaved):
    - dynamic-offset DMA stages the 256-row slab [128p, 2slot, 128f]
    - gpsimd.partition_broadcast replicates the int8 local-idx row
    - DVE is_equal vs per-partition constants -> two one-hot f16 tiles
    - PE: psum[f,e] = slab0.T@E0 + slab1.T@E1 (+ We.T@edge_attr on pass A)
    - ACT copies psum -> fp16 staging, feature-major DMA store
  Host: un-permute the two partial outputs (f32) and add.
"""

import sys
import numpy as np

if "/opt/trn_rl_repo" not in sys.path:
    sys.path.insert(0, "/opt/trn_rl_repo")

P = 128
CH = 512          # edge columns per chunk (PSUM bank = 512 f32)
SLAB = 256        # node rows staged per chunk (2 matmul slots)
N_CORES = 8
N_NODES = 50000
N_IN_NODE = 128
N_IN_EDGE = 64
N_OUT = 128
N_EDGES = 1000000
E_CORE = N_EDGES // N_CORES            # 125000
NODES_PAD = (N_NODES + P - 1) // P * P  # 50176
A_TILES = NODES_PAD // P               # 392


def build_program(n_ch, nodes_pad=NODES_PAD, n_cores=N_CORES):
    """n_ch: chunks per pass (same for all cores; host pads to this)."""
    import concourse.mybir as mybir
    import concourse.tile as tile
    from concourse import bacc
    from concourse import bass as cbass

    f32 = mybir.dt.float32
    f16 = mybir.dt.float16
    i8 = mybir.dt.int8
    i32 = mybir.dt.int32
    EQ = mybir.AluOpType.is_equal
    ds = cbass.ds

    e_dev = n_ch * CH
    a_tiles = nodes_pad // P

    nc = bacc.Bacc("TRN2", target_bir_lowering=False, debug=False,
                   num_devices=n_cores)

    xT_d = nc.dram_tensor("xT", [P, nodes_pad], f16, kind="ExternalInput").ap()
    wsT_d = nc.dram_tensor("wsT", [P, P], f16, kind="ExternalInput").ap()
    wdT_d = nc.dram_tensor("wdT", [P, P], f16, kind="ExternalInput").ap()
    weT_d = nc.dram_tensor("weT", [N_IN_EDGE, P], f16, kind="ExternalInput").ap()
    bias_d = nc.dram_tensor("bias", [P, P], f32, kind="ExternalInput").ap()
    pidx_d = nc.dram_tensor("pidx", [P, 2 * CH], f16, kind="ExternalInput").ap()
    basA_d = nc.dram_tensor("basA", [1, n_ch], i32, kind="ExternalInput").ap()
    basB_d = nc.dram_tensor("basB", [1, n_ch], i32, kind="ExternalInput").ap()
    liA_d = nc.dram_tensor("liA", [1, e_dev], f16, kind="ExternalInput").ap()
    liB_d = nc.dram_tensor("liB", [1, e_dev], f16, kind="ExternalInput").ap()
    eatA_d = nc.dram_tensor("eatA", [N_IN_EDGE, e_dev], f16, kind="ExternalInput").ap()
    outA_d = nc.dram_tensor("outA", [P, e_dev], f16, kind="ExternalOutput").ap()
    outB_d = nc.dram_tensor("outB", [P, e_dev], f16, kind="ExternalOutput").ap()
    ys_d = nc.dram_tensor("ys", [nodes_pad, P], f16, kind="Internal").ap()
    yd_d = nc.dram_tensor("yd", [nodes_pad, P], f16, kind="Internal").ap()

    GRP = 16   # node tiles per phase-A group
    LGRP = 8   # chunks per li-row load
    OGRP = 4   # chunks per output staging group

    with tile.TileContext(nc) as tc:
        with tc.tile_pool(name="static", bufs=1) as spool:
            wsT_sb = spool.tile([P, P], f16)
            nc.sync.dma_start(wsT_sb[:], wsT_d[:, :])
            wdT_sb = spool.tile([P, P], f16)
            nc.sync.dma_start(wdT_sb[:], wdT_d[:, :])
            weT_sb = spool.tile([N_IN_EDGE, P], f16)
            nc.sync.dma_start(weT_sb[:], weT_d[:, :])
            bias_sb = spool.tile([P, P], f32)
            nc.sync.dma_start(bias_sb[:], bias_d[:, :])
            pidx_sb = spool.tile([P, 2 * CH], f16)
            nc.sync.dma_start(pidx_sb[:], pidx_d[:, :])
            basA_sb = spool.tile([1, n_ch], i32)
            nc.sync.dma_start(basA_sb[:], basA_d[:, :])
            basB_sb = spool.tile([1, n_ch], i32)
            nc.sync.dma_start(basB_sb[:], basB_d[:, :])

            # ---- Phase A: Ys = x@Wsrc.T + b, Yd = x@Wdst.T (node-major) ----
            with tc.tile_pool(name="pa", bufs=2) as papool, \
                 tc.tile_pool(name="paps", bufs=4, space="PSUM") as paps:
                for g0 in range(0, a_tiles, GRP):
                    gn = min(GRP, a_tiles - g0)
                    xt = papool.tile([P, GRP * P], f16, tag="xt")
                    nc.sync.dma_start(xt[:, :gn * P],
                                      xT_d[:, g0 * P:(g0 + gn) * P])
                    ysb = papool.tile([P, GRP, P], f16, tag="ysb")
                    ydb = papool.tile([P, GRP, P], f16, tag="ydb")
                    for t in range(gn):
                        ps = paps.tile([P, 2 * P], f32, tag="ps")
                        nc.tensor.matmul(ps[:, 0:P],
                                         lhsT=xt[:, t * P:(t + 1) * P],
                                         rhs=wsT_sb[:], start=True, stop=True)
                        nc.tensor.matmul(ps[:, P:2 * P],
                                         lhsT=xt[:, t * P:(t + 1) * P],
                                         rhs=wdT_sb[:], start=True, stop=True)
                        nc.vector.tensor_add(ysb[:, t, :], ps[:, 0:P], bias_sb[:])
                        nc.scalar.copy(ydb[:, t, :], ps[:, P:2 * P])
                    ys_rows = ys_d[g0 * P:(g0 + gn) * P, :].rearrange(
                        "(t p) f -> p t f", p=P)
                    yd_rows = yd_d[g0 * P:(g0 + gn) * P, :].rearrange(
                        "(t p) f -> p t f", p=P)
                    nc.sync.dma_start(ys_rows[:, :, :], ysb[:, :gn, :])
                    nc.gpsimd.dma_start(yd_rows[:, :, :], ydb[:, :gn, :])

            tc.strict_bb_all_engine_barrier()

            # ---- Passes A (src) and B (dst), interleaved chunk loop ----
            with tc.tile_pool(name="pb", bufs=3) as bpool, \
                 tc.tile_pool(name="bps", bufs=4, space="PSUM") as bps:
                passes = [
                    ("A", basA_sb, liA_d, ys_d, outA_d, True),
                    ("B", basB_sb, liB_d, yd_d, outB_d, False),
                ]
                li_rows = {}
                eat_rows = {}
                outst = {}
                # batched base loads: 2 banks x 8 regs on sync; one reg_load
                # per 4 chunks per pass refills a bank while the other drains
                RB = 4  # chunks per refill per pass
                sregs = [[nc.sync.alloc_register(f"sb{h}_{j}")
                          for j in range(2 * RB)] for h in range(2)]
                for k in range(n_ch):
                    for (tagp, bas_sb, li_d, tab_d, out_d, has_z) in passes:
                        if k % LGRP == 0:
                            lw = min(LGRP, n_ch - k) * CH
                            lr = bpool.tile([P, LGRP * CH], f16,
                                            tag=f"li{tagp}")
                            nc.gpsimd.dma_start(
                                lr[:, :lw],
                                li_d[0:1, k * CH:k * CH + lw].to_broadcast(
                                    [P, lw]))
                            li_rows[tagp] = lr
                            if has_z:
                                er = bpool.tile([N_IN_EDGE, LGRP * CH], f16,
                                                tag="eat")
                                nc.gpsimd.dma_start(
                                    er[:, :lw],
                                    eatA_d[:, k * CH:k * CH + lw])
                                eat_rows[tagp] = er
                        eng = nc.sync
                        pi = 0 if tagp == "A" else 1
                        if k % RB == 0:
                            bank = sregs[(k // RB) % 2]
                            rn = min(RB, n_ch - k)
                            nc.sync.reg_load(
                                bank[pi * RB:pi * RB + rn],
                                bas_sb[0:1, k:k + rn])
                        r = sregs[(k // RB) % 2][pi * RB + k % RB]
                        base = eng.snap(r, min_val=0,
                                        max_val=nodes_pad - SLAB)
                        slab = bpool.tile([P, 2, P], f16, tag=f"slab{tagp}")
                        eng.dma_start(
                            slab[:, :, :],
                            tab_d[ds(base, SLAB), :].rearrange(
                                "(s p) f -> p s f", p=P))
                        libc = li_rows[tagp][
                            :, (k % LGRP) * CH:(k % LGRP + 1) * CH]
                        eT0 = bpool.tile([P, CH], f16, tag=f"eT0{tagp}")
                        nc.vector.tensor_tensor(
                            eT0[:, :], libc, pidx_sb[:, 0:CH], op=EQ)
                        eT1 = bpool.tile([P, CH], f16, tag=f"eT1{tagp}")
                        nc.vector.tensor_tensor(
                            eT1[:, :], libc, pidx_sb[:, CH:2 * CH], op=EQ)
                        po = bps.tile([P, CH], f32, tag=f"po{tagp}")
                        nc.tensor.matmul(po[:, :], lhsT=slab[:, 0, :],
                                         rhs=eT0[:, :], start=True, stop=False)
                        nc.tensor.matmul(po[:, :], lhsT=slab[:, 1, :],
                                         rhs=eT1[:, :], start=False,
                                         stop=not has_z)
                        if has_z:
                            nc.tensor.matmul(
                                po[:, :], lhsT=weT_sb[:],
                                rhs=eat_rows[tagp][
                                    :, (k % LGRP) * CH:(k % LGRP + 1) * CH],
                                start=False, stop=True)
                        if k % OGRP == 0:
                            ot = bpool.tile([P, OGRP * CH], f16,
                                            tag=f"outst{tagp}",
                                            name=f"outst{tagp}_{k}")
                            outst[tagp] = ot
                        nc.scalar.copy(
                            outst[tagp][:, (k % OGRP) * CH:(k % OGRP + 1) * CH],
                            po[:, :])
                        if k % OGRP == OGRP - 1 or k == n_ch - 1:
                            k0 = k - (k % OGRP)
                            nc.sync.dma_start(
                                out_d[:, k0 * CH:(k + 1) * CH],
                                outst[tagp][:, :(k % OGRP + 1) * CH])

    nc.compile()
    return nc


def _plan_pass(key_sorted):
    """Greedy chunking of a sorted node-id stream.

    Returns list of (start, count, base): count <= CH edges starting at
    `start` whose ids fit in [base, base+SLAB)."""
    n = len(key_sorted)
    chunks = []
    i = 0
    while i < n:
        base = int(key_sorted[i])
        base = min(base, NODES_PAD - SLAB)
        j = min(i + CH, n)
        # first index whose id falls outside the slab
        j = i + int(np.searchsorted(key_sorted[i:j], base + SLAB, side="left"))
        chunks.append((i, j - i, base))
        i = j
    return chunks


def prep_core(src, dst, edge_attr_core):
    """Per-core host prep. Returns dict of device arrays + colmaps."""
    plans = {}
    for tagp, key in (("A", src), ("B", dst)):
        perm = np.argsort(key, kind="stable")
        ks = key[perm].astype(np.int64)
        chunks = _plan_pass(ks)
        plans[tagp] = (perm, ks, chunks)
    return plans


def pack_core(plans, edge_attr_core, n_ch):
    e_dev = n_ch * CH
    dev = {}
    colmaps = {}
    for tagp, (perm, ks, chunks) in plans.items():
        li = np.zeros(e_dev, dtype=np.float16)
        bases = np.zeros(n_ch, dtype=np.int32)
        colmap = np.full(e_dev, -1, dtype=np.int64)
        for c, (s, cnt, base) in enumerate(chunks):
            bases[c] = base
            sl = slice(c * CH, c * CH + cnt)
            li[sl] = (ks[s:s + cnt] - base).astype(np.float16)
            colmap[sl] = perm[s:s + cnt]
        dev[f"li{tagp}"] = li[None, :]
        dev[f"bas{tagp}"] = bases[None, :]
        colmaps[tagp] = colmap
        if tagp == "A":
            eat = np.zeros((N_IN_EDGE, e_dev), dtype=np.float16)
            valid = colmap >= 0
            eat[:, valid] = edge_attr_core[colmap[valid]].astype(np.float16).T
            dev["eatA"] = eat
    return dev, colmaps


def prep_inputs(x, edge_index, edge_attr, W, b):
    """Host-side prep: shard + sort + pack. Returns (in_maps, colmaps, n_ch)."""
    x = np.asarray(x, dtype=np.float32)
    edge_index = np.asarray(edge_index)
    edge_attr = np.asarray(edge_attr, dtype=np.float32)
    W = np.asarray(W, dtype=np.float32)
    b = np.asarray(b, dtype=np.float32)

    xT = np.zeros((P, NODES_PAD), dtype=np.float16)
    xT[:, :N_NODES] = x.astype(np.float16).T
    wsT = np.ascontiguousarray(W[:, :P].T).astype(np.float16)
    wdT = np.ascontiguousarray(W[:, P:2 * P].T).astype(np.float16)
    weT = np.ascontiguousarray(W[:, 2 * P:].T).astype(np.float16)
    bias_rep = np.ascontiguousarray(
        np.tile(b[None, :].astype(np.float32), (P, 1)))
    pcol = np.arange(P, dtype=np.float16)[:, None]
    pidx = np.concatenate([np.tile(pcol, (1, CH)),
                           np.tile(pcol + 128, (1, CH))],
                          axis=1).astype(np.float16)

    src = np.ascontiguousarray(edge_index[0]).astype(np.int64)
    dst = np.ascontiguousarray(edge_index[1]).astype(np.int64)

    core_plans = []
    n_ch = 0
    for c in range(N_CORES):
        lo, hi = c * E_CORE, (c + 1) * E_CORE
        plans = prep_core(src[lo:hi], dst[lo:hi], None)
        for tagp in ("A", "B"):
            n_ch = max(n_ch, len(plans[tagp][2]))
        core_plans.append(plans)

    in_maps = []
    all_colmaps = []
    for c in range(N_CORES):
        lo, hi = c * E_CORE, (c + 1) * E_CORE
        dev, colmaps = pack_core(core_plans[c], edge_attr[lo:hi], n_ch)
        dev.update({
            "xT": xT, "wsT": wsT, "wdT": wdT, "weT": weT,
            "bias": bias_rep, "pidx": pidx,
        })
        in_maps.append(dev)
        all_colmaps.append(colmaps)
    return in_maps, all_colmaps, n_ch


_NC_CACHE = {}


def _get_program(n_ch):
    if n_ch not in _NC_CACHE:
        _NC_CACHE[n_ch] = build_program(n_ch)
    return _NC_CACHE[n_ch]


def run_on_hw(in_maps, nc=None, trace=False, n_cores=N_CORES):
    from concourse import bass_utils
    if nc is None:
        raise ValueError("pass nc")
    kw = {}
    if trace:
        _install_profile_hook(bass_utils)
        kw["trace"] = True
    res = bass_utils.run_bass_kernel_spmd(
        nc, in_maps, core_ids=list(range(n_cores)), **kw)
    return res


def _install_profile_hook(bass_utils):
    """Inject the NTFF profile hook missing from this image's antenv."""
    import types
    if "antenv.axon_hooks" in sys.modules:
        return
    try:
        from trn_agent_boot.trn_boot import _ntff_profile_via_ctypes
        hook = _ntff_profile_via_ctypes("/opt/axon/libaxon_pjrt.so")
    except Exception:
        hook = None
    mod = types.ModuleType("antenv.axon_hooks")
    mod.get_axon_ntff_profile_hook = lambda: hook
    mod.set_axon_ntff_profile_hook = lambda h: None
    sys.modules["antenv.axon_hooks"] = mod
    bass_utils.upload_artifacts = lambda tmpdir: f"file://{tmpdir}"


def combine_outputs(res, all_colmaps):
    out = np.zeros((N_EDGES, N_OUT), dtype=np.float32)
    for c in range(N_CORES):
        lo = c * E_CORE
        cmA = all_colmaps[c]["A"]
        cmB = all_colmaps[c]["B"]
        outA = np.asarray(res.results[c]["outA"])  # [128, e_dev] f16
        outB = np.asarray(res.results[c]["outB"])
        vA = cmA >= 0
        vB = cmB >= 0
        out[lo + cmA[vA]] = outA[:, vA].T.astype(np.float32)
        out[lo + cmB[vB]] += outB[:, vB].T.astype(np.float32)
    return out


def kernel(x, edge_index, edge_attr, W, b):
    in_maps, all_colmaps, n_ch = prep_inputs(x, edge_index, edge_attr, W, b)
    nc = _get_program(n_ch)
    res = run_on_hw(in_maps, nc=nc)
    return combine_outputs(res, all_colmaps)
